# revision 20
# baseline (speedup 1.0000x reference)
"""Trainium2 Bass kernel: GQA attention over packed ragged sequences.

Sharding: tensor-parallel over heads across 8 NeuronCores. Core c owns
q-heads [4c, 4c+4) and kv-head c. wq/wk/wv are sharded by head rows,
wo by columns; each core computes a full [S, DIM] partial of the output
projection and the partials are summed on the host (row-parallel
all-reduce of wo).

Layout choices:
 - All projections computed in transposed layout (feature dim on SBUF
   partitions, tokens on free dim) so the attention contractions have
   their contraction dim on partitions.
 - RoPE pairs are de-interleaved by permuting wq/wk rows on the host
   (evens then odds per head), turning RoPE into the rotate-half form:
   out = q*cos2 + roll64(q)*sin2 with the sign folded into sin2.
 - Data path is bf16 (fp32 PSUM accumulate): halves DMA + DVE cost,
   enables fast weight load, full PE rate. Output partials are stored
   bf16 and summed fp32 on host.
 - Single allocation epoch (everything fits in SBUF at bf16): no pool
   transitions. Attention for a sequence is emitted between projection
   quarters as soon as its tokens are RoPE'd; output-projection pieces
   of finished sequences are fed into attention's dependency bubbles.
"""

import os
from collections import deque
from contextlib import ExitStack
import numpy as np
import ml_dtypes

import concourse.bass as bass
import concourse.mybir as mybir
from concourse.tile import TileContext, add_dep_helper
from concourse.bass_utils import run_bass_kernel_spmd

F32 = mybir.dt.float32
BF16 = mybir.dt.bfloat16
AX = mybir.AxisListType
ALU = mybir.AluOpType
ACT = mybir.ActivationFunctionType

H, KVH, D, DIM = 32, 8, 128, 4096
NCORES = 8
HPC = H // NCORES          # q heads per core
S = 2048                   # total packed tokens
SCALE = D ** -0.5
P = 128                    # partition count / tile edge
KC = DIM // P              # contraction chunks for qkv projections
TQ, TW = 4, 512            # token quarters for projection phase
NXT = 16                   # x stream tiles (8 per alternating half)
NEG = -1.0e30

LAST_RESULTS = None        # BassKernelResults of the most recent run


def _build(seq_tiles):
    """Build the per-core Bass program. seq_tiles: tiles (of 128 tokens)
    per packed sequence, e.g. (4, 6, 2, 4).

    Sync-wait discipline (walrus caps: PE matmul LW = 1 wait, DMA = 2):
    - every DMA-produced tile is first read by a tiny PE "touch" matmul
      into a persistent [1,2] psum scratch (absorbs the DMA wait);
    - all PSUM lives in persistent tiles from one global pool;
    - _prune_waits recomputes vector clocks and drops redundant waits.
    """
    LMAX = max(seq_tiles) * P
    nseq = len(seq_tiles)
    assert LMAX <= 1024 and sum(seq_tiles) * P == S
    t0s = np.concatenate([[0], np.cumsum(seq_tiles)]).astype(int)
    # tq after whose epilogue sequence s is fully projected+RoPE'd
    ready_tq = [(int(t0s[s + 1]) * P + TW - 1) // TW - 1 for s in range(nseq)]

    nc = bass.Bass()

    xT = nc.dram_tensor("xT", [DIM, S], BF16, kind="ExternalInput")
    wqT = nc.dram_tensor("wqT", [DIM, HPC * D], BF16, kind="ExternalInput")
    wkT = nc.dram_tensor("wkT", [DIM, D], BF16, kind="ExternalInput")
    wvT = nc.dram_tensor("wvT", [DIM, D], BF16, kind="ExternalInput")
    woT = nc.dram_tensor("woT", [HPC * D, DIM], BF16, kind="ExternalInput")
    cos2 = nc.dram_tensor("cos2", [P, S], BF16, kind="ExternalInput")
    sin2 = nc.dram_tensor("sin2", [P, S], BF16, kind="ExternalInput")
    trim = nc.dram_tensor("trim", [P, P], F32, kind="ExternalInput")
    identh = nc.dram_tensor("identh", [P, P], BF16, kind="ExternalInput")
    out_d = nc.dram_tensor("out", [S, DIM], BF16, kind="ExternalOutput")

    with TileContext(nc) as tc:
        with tc.tile_pool(name="glob", bufs=1) as gp, \
             tc.tile_pool(name="globps", space="PSUM", bufs=1) as gpp, \
             tc.tile_pool(name="qkv", bufs=1, side="right") as qkvp, \
             tc.tile_pool(name="wA", bufs=1) as wp, \
             tc.tile_pool(name="xA", bufs=1) as xp, \
             tc.tile_pool(name="csA", bufs=1) as csp, \
             tc.tile_pool(name="ropeA", bufs=1) as rp, \
             tc.tile_pool(name="attP", bufs=1) as attp, \
             tc.tile_pool(name="woP", bufs=1) as wop, \
             tc.tile_pool(name="attw", bufs=1) as ap_:
            trimt = gp.tile([P, P], F32, name="trimt")
            ident = gp.tile([P, P], BF16, name="ident")
            nc.sync.dma_start(out=trimt[:, :], in_=trim[:, :])
            nc.sync.dma_start(out=ident[:, :], in_=identh[:, :])

            # q/k/v residents
            qTt = qkvp.tile([P, HPC * S], BF16, name="qTt")  # per-head [d, tok]
            kTt = qkvp.tile([P, S], BF16, name="kTt")        # [d, tok]
            vt = qkvp.tile([P, S], BF16, name="vt")          # [tok%128, blk*128+d]

            # PSUM: 3x 2-bank fp32 + two bf16 transpose banks (6, 7) with
            # an fp32 touch target aliased into bigV's tail (touches only
            # run while transpose slots hold dead data)
            big0 = gpp.tile([P, 1024], F32, name="big0")
            big1 = gpp.tile([P, 1024], F32, name="big1")
            big2 = gpp.tile([P, 1024], F32, name="big2")
            bigU = gpp.tile([P, 1024], BF16, name="bigU")
            bigV = gpp.tile([P, 1024], BF16, name="bigV")
            tps = bigV[0:1, 768:1024].bitcast(F32)

            def touch(t):
                # N=1 matmuls fail walrus's ISA check; use a [K,1]x[K,2] probe
                return nc.tensor.matmul(tps[0:1, 0:2], t[:, 0:1], t[:, 0:2],
                                        start=True, stop=True)

            touch(ident)  # absorb the identity DMA wait once

            # ---- projection-phase tiles ----
            cos2t = csp.tile([P, S], BF16, name="cos2t")
            sin2t = csp.tile([P, S], BF16, name="sin2t")
            xts = [xp.tile([P, TW], BF16, name=f"xts{i}") for i in range(NXT)]
            wq_c, wk_c, wv_c = [], [], []

            psq = [big0[:, 0:TW], big0[:, TW:2 * TW],
                   big1[:, 0:TW], big1[:, TW:2 * TW]]
            psk = big2[:, 0:TW]
            psv = big2[:, TW:2 * TW]
            pstA2 = [bigU[:, 0:P], bigV[:, 0:P]]

            traw6 = [rp.tile([P, TW], BF16, name=f"traw{i}") for i in range(5)]
            rot2 = [rp.tile([P, TW], BF16, name=f"rot{i}") for i in range(2)]
            vtmp2 = [rp.tile([P, TW], BF16, name=f"vtmp{i}") for i in range(2)]

            # ---- attention / wo tiles ----
            attT = attp.tile([P, HPC * S], BF16, name="attTt")
            Ptm2 = [ap_.tile([P, LMAX], BF16, name=f"Ptmp{i}") for i in range(4)]
            strips = [[ap_.tile([P, LMAX], BF16, name=f"strip{b}_{kb}")
                       for kb in range(max(seq_tiles))] for b in range(3)]
            ones2 = ap_.tile([P, 2], BF16, name="ones2")
            nc.vector.memset(ones2[:, :], 1.0)
            onesrow = ap_.tile([1, P], BF16, name="onesrow")
            nc.vector.memset(onesrow[:, :], 1.0)
            rrow2 = [ap_.tile([1, LMAX], BF16, name=f"rrow{i}") for i in range(2)]
            rrbc2 = [ap_.tile([P, LMAX], F32, name=f"rrbc{i}") for i in range(2)]
            ots1 = [ap_.tile([P, 512], BF16, name=f"ots1_{i}") for i in range(4)]

            ps_s4 = [big0[:, 0:512], big0[:, 512:1024],
                     big1[:, 0:512], big1[:, 512:1024]]
            ps_sbig = [big0[:, 0:LMAX], big1[:, 0:LMAX]]
            ps_t2 = [bigU[:, 512:512 + P], bigV[:, 256:256 + P]]

            # full wo resident (bf16 leaves room); DMA'd during tq1
            woh = [wop.tile([P, DIM], BF16, name=f"woh{f}") for f in range(HPC)]
            woh_t = [None] * HPC

            # ---- shared emission state ----
            # it: psum-slot rotation (shared by scores + wo pieces; slots
            #     are freed within one group so distance 4 is safe).
            # sc: scores-only rotation for Ptmp/Pt/se/rr, whose lifetime
            #     spans the 2-tile transpose lag — feeding wo pieces into
            #     `it` must not shorten their reuse distance.
            st_ = {"it": 0, "sc": 0, "hs": 0, "ctile": 0}
            wo_feed = deque()   # pending wo-piece callables
            feed_credit = [0.0]

            def emit_wo_piece():
                if wo_feed:
                    wo_feed.popleft()()

            def feed(rate):
                feed_credit[0] += rate
                while feed_credit[0] >= 1.0 and wo_feed:
                    feed_credit[0] -= 1.0
                    emit_wo_piece()

            def make_wo_piece(mt, nch):
                msl = slice(mt * P, (mt + 1) * P)
                nsl = slice(nch * 512, (nch + 1) * 512)

                def piece():
                    ps = ps_s4[st_["it"] % 4]
                    st_["it"] += 1
                    for f in range(HPC):
                        mi = nc.tensor.matmul(
                            ps, attT[:, f * S + mt * P: f * S + (mt + 1) * P],
                            woh[f][:, nsl], start=(f == 0), stop=(f == HPC - 1))
                        if woh_t[f] is not None:
                            add_dep_helper(mi.ins, woh_t[f].ins, sync=False,
                                           reason="woh touch first")
                            woh_t[f] = None
                    ot = ots1[st_["ctile"] % 4]
                    st_["ctile"] += 1
                    nc.scalar.copy(ot[:, :], ps)
                    nc.sync.dma_start(out=out_d[msl, nsl], in_=ot[:, :])
                return piece

            def enqueue_wo(s):
                for mt in range(int(t0s[s]), int(t0s[s + 1])):
                    for nch in range(DIM // 512):
                        wo_feed.append(make_wo_piece(mt, nch))

            def attn_seq(s, rate):
                """Emit attention for sequence s. Yields between PE groups;
                interleaves wo-feed pieces at the given rate per group.
                Software-pipelined: transposes for a tile are emitted two
                score-tiles later so PE has independent work while the
                exp/normalize chain completes."""
                T = int(seq_tiles[s])
                kb0 = int(t0s[s]) * P
                LAGN = 2

                def scores_tile(h, qt):
                    L = (qt + 1) * P
                    q0 = kb0 + qt * P
                    ps_s = (ps_s4[st_["it"] % 4] if L <= 512
                            else ps_sbig[st_["it"] % 2])
                    st_["it"] += 1
                    b = st_["sc"] % 4
                    st_["sc"] += 1
                    Ptmp = Ptm2[b]
                    qsl = qTt[:, h * S + q0: h * S + q0 + P]
                    off = 0
                    while off < L:
                        w = min(512, L - off)
                        nc.tensor.matmul(ps_s[:, off:off + w], qsl,
                                         kTt[:, kb0 + off: kb0 + off + w],
                                         start=True, stop=True)
                        off += w
                    nc.vector.tensor_add(ps_s[:, L - P:L], ps_s[:, L - P:L],
                                         trimt[:, :])
                    # no max-subtraction: |scale*s| < ~25 keeps exp finite.
                    # Normalization is deferred: strips hold raw exp; the
                    # denominator is recovered from the strips by a ones-
                    # matmul and folded into the PV->attT copy.
                    nc.scalar.activation(Ptmp[:, :L], ps_s[:, :L], ACT.Exp,
                                         scale=SCALE)
                    return b

                def transposes_tile(sb, qt, b):
                    Ptmp = Ptm2[b]
                    for kb in range(qt + 1):
                        ps_t = ps_t2[kb % 2]
                        nc.tensor.transpose(ps_t,
                                            Ptmp[:, kb * P:(kb + 1) * P],
                                            ident[:, :])
                        nc.vector.tensor_copy(
                            sb[kb][:, qt * P:(qt + 1) * P], ps_t)

                for h in range(HPC):
                    sb = strips[st_["hs"] % 3]
                    rrow = rrow2[st_["hs"] % 2]
                    rrbc = rrbc2[st_["hs"] % 2]
                    st_["hs"] += 1
                    pend = deque()
                    for qt in range(T):
                        b = scores_tile(h, qt)
                        pend.append((qt, b))
                        feed(rate)
                        yield
                        if len(pend) > LAGN:
                            qt_, b_ = pend.popleft()
                            transposes_tile(sb, qt_, b_)
                            feed(rate)
                            yield
                    while pend:
                        qt_, b_ = pend.popleft()
                        transposes_tile(sb, qt_, b_)
                        feed(rate)
                        yield
                    W = T * P
                    # softmax denominators: colsums of the strips via a
                    # tiny ones-matmul, reciprocal, then broadcast to all
                    # partitions with a K=1 outer-product matmul; folded
                    # into the PV->attT copy below
                    for c0 in range(0, W, 512):
                        c1 = min(c0 + 512, W)
                        seb = ps_s4[st_["it"] % 4]
                        st_["it"] += 1
                        nkb = c1 // P
                        for kb in range(nkb):
                            r0 = max(c0, kb * P)
                            nc.tensor.matmul(seb[0:2, r0 - c0:c1 - c0],
                                             ones2[:, :], sb[kb][:, r0:c1],
                                             start=(kb == 0),
                                             stop=(kb == nkb - 1))
                        with nc.allow_low_precision(
                                reason="bf16 1/se feeds a bf16 matmul; "
                                       "0.4% on softmax scale is in budget"):
                            nc.vector.reciprocal(rrow[0:1, c0:c1],
                                                 seb[0:1, 0:c1 - c0])
                    for c0 in range(0, W, 512):
                        c1 = min(c0 + 512, W)
                        ob_ = ps_s4[st_["it"] % 4]
                        st_["it"] += 1
                        nc.tensor.matmul(ob_[:, 0:c1 - c0], onesrow[0:1, :],
                                         rrow[0:1, c0:c1],
                                         start=True, stop=True)
                        nc.scalar.copy(rrbc[:, c0:c1], ob_[:, 0:c1 - c0])
                    # PV over the transposed strips: out[d, q-range]
                    for c0 in range(0, W, 512):
                        c1 = min(c0 + 512, W)
                        nkb = c1 // P
                        for kb in range(nkb):
                            r0 = max(c0, kb * P)
                            kt0 = kb0 + kb * P
                            nc.tensor.matmul(big2[:, r0:c1],
                                             vt[:, kt0:kt0 + P],
                                             sb[kb][:, r0:c1],
                                             start=(kb == 0),
                                             stop=(kb == nkb - 1))
                        feed(rate)
                        yield
                    nc.vector.tensor_mul(
                        attT[:, h * S + kb0: h * S + kb0 + W], big2[:, 0:W],
                        rrbc[:, 0:W])
                    feed(rate)
                    yield

            def run_gen(g):
                for _ in g:
                    pass

            def rope_finish(traw, rot, dest, tsl):
                """dest = traw*cos2 + roll64(traw)*sin2 (sign in sin2)."""
                nc.vector.tensor_mul(rot[0:64, :], traw[64:128, :],
                                     sin2t[64:128, tsl])
                nc.vector.tensor_mul(rot[64:128, :], traw[0:64, :],
                                     sin2t[0:64, tsl])
                nc.vector.tensor_mul(traw[:, :], traw[:, :], cos2t[:, tsl])
                nc.vector.tensor_add(dest, traw[:, :], rot[:, :])

            # ================= main emission =================
            for tq in range(TQ):
                tsl = slice(tq * TW, (tq + 1) * TW)
                for k in range(KC):
                    xt = xts[k % 8 + 8 * (tq % 2)]
                    nc.sync.dma_start(out=xt[:, :],
                                      in_=xT[k * P:(k + 1) * P, tsl])
                    if tq == 0:
                        wq_k = wp.tile([P, HPC * D], BF16, name=f"wq{k}")
                        wk_k = wp.tile([P, D], BF16, name=f"wk{k}")
                        wv_k = wp.tile([P, D], BF16, name=f"wv{k}")
                        ksl = slice(k * P, (k + 1) * P)
                        nc.sync.dma_start(out=wq_k[:, :], in_=wqT[ksl, :])
                        nc.sync.dma_start(out=wk_k[:, :], in_=wkT[ksl, :])
                        nc.sync.dma_start(out=wv_k[:, :], in_=wvT[ksl, :])
                        wq_c.append(wq_k)
                        wk_c.append(wk_k)
                        wv_c.append(wv_k)
                        if k == 16:
                            nc.sync.dma_start(out=cos2t[:, :], in_=cos2[:, :])
                            nc.sync.dma_start(out=sin2t[:, :], in_=sin2[:, :])
                    if tq == 1 and k % 8 == 0:
                        # stream the wo slab in during tq1 (x traffic halves
                        # after tq0's weights are done)
                        f = k // 8
                        nc.sync.dma_start(out=woh[f][:, :],
                                          in_=woT[f * P:(f + 1) * P, :])
                    need_touch = (tq == 0) or (k == 0)
                    ti = touch(xt) if need_touch else None
                    st, sp = (k == 0), (k == KC - 1)
                    mms = []
                    for h in range(HPC):
                        mms.append(nc.tensor.matmul(
                            psq[h], wq_c[k][:, h * D:(h + 1) * D],
                            xt[:, :], start=st, stop=sp))
                    mms.append(nc.tensor.matmul(
                        psk, wk_c[k][:, :], xt[:, :], start=st, stop=sp))
                    mms.append(nc.tensor.matmul(
                        psv, wv_c[k][:, :], xt[:, :], start=st, stop=sp))
                    if ti is not None:
                        for mi in mms:
                            add_dep_helper(mi.ins, ti.ins, sync=False,
                                           reason="xt touch first")
                # epilogue: drain psum banks with copies split across ACT
                # and DVE (banks 0-3 are needed by the attention segment
                # right after); RoPE itself is deferred past the segment —
                # its results aren't needed until a later segment.
                vtmp = vtmp2[tq % 2]
                nc.scalar.copy(vtmp[:, :], psv)
                nc.scalar.copy(traw6[0][:, :], psq[0])
                nc.vector.tensor_copy(traw6[1][:, :], psq[1])
                nc.scalar.copy(traw6[2][:, :], psq[2])
                nc.vector.tensor_copy(traw6[3][:, :], psq[3])
                for j in range(TW // P):
                    pstA = pstA2[j % 2]
                    nc.tensor.transpose(pstA, vtmp[:, j * P:(j + 1) * P],
                                        ident[:, :])
                    tok = tq * TW + j * P
                    nc.scalar.copy(vt[:, tok:tok + P], pstA)
                nc.vector.tensor_copy(traw6[4][:, :], psk)

                def emit_rope(tq_):
                    tsl_ = slice(tq_ * TW, (tq_ + 1) * TW)
                    for h in range(HPC):
                        rope_finish(
                            traw6[h], rot2[h % 2],
                            qTt[:, h * S + tq_ * TW: h * S + (tq_ + 1) * TW],
                            tsl_)
                    rope_finish(traw6[4], rot2[0], kTt[:, tsl_], tsl_)

                # ---- post-tq attention / wo segments ----
                if tq == 0:
                    emit_rope(0)
                if tq == 1:
                    for s in range(nseq):
                        if ready_tq[s] <= 0:
                            run_gen(attn_seq(s, 0.0))
                            enqueue_wo(s)
                    emit_rope(1)
                if tq == 2:
                    for f in range(HPC):
                        woh_t[f] = touch(woh[f])
                    # drain most of seq0's wo here; keep some as feed for
                    # the first post-tq3 attention segment
                    while len(wo_feed) > 24:
                        emit_wo_piece()
                    emit_rope(2)
            # after the last tq: remaining sequences, feeding wo pieces of
            # finished sequences into attention bubbles, then drain.
            # tq3's RoPE is emitted after the first segment (it is only
            # needed by the last sequence's attention).
            done = [s for s in range(nseq) if ready_tq[s] <= 0]
            rope3 = [False]
            for s in range(nseq):
                if s in done:
                    continue
                npieces = max(1, len(wo_feed))
                ngroups = (2 * int(seq_tiles[s]) + 3) * HPC
                rate = min(3.0, npieces / ngroups)
                run_gen(attn_seq(s, rate))
                enqueue_wo(s)
                if not rope3[0]:
                    emit_rope(3)
                    rope3[0] = True
            if not rope3[0]:
                emit_rope(3)
            while wo_feed:
                emit_wo_piece()

    _prune_waits(nc)
    return nc


def _prune_waits(nc):
    """Fit instructions into walrus's per-instruction sync-command budget
    (~2 commands: waits + updates; matmul LW and DMA take 1 wait).

    Tile's stage-1B emits each instruction's full required vector clock as
    waits when the executing proc hasn't observed the ticks, without
    transitive minimization. We recompute exact vector clocks over the
    emitted sem graph (procs execute in program order; engines and DMA
    lanes complete in FIFO order), drop waits implied by the proc's
    predecessor or by other kept waits' grants, and move any genuine
    overflow onto injected same-engine NoOps just before the instruction.
    """
    import concourse.mybir as _mybir

    f = nc.m.functions[0]

    CAP = {}
    SKIP = {"NoOp", "EventSemaphore", "AllEngineBarrier", "Halt"}
    DEFAULT_CAP = 1

    def join(a, b):
        for k, v in b.items():
            if a.get(k, -1) < v:
                a[k] = v
        return a

    sem_hist = {}    # sem id -> list of (cum_value_after, vec_of_updater)
    sem_cum = {}     # sem id -> cumulative value
    proc_vec = {}    # proc key -> vector of last completed inst on proc
    nop_n = [0]

    def proc_of(i):
        si = i.sync_info
        if i.opcode in ("DMACopy", "DMATranspose") and si and si.on_update:
            return ("sem", si.on_update[0].id)
        return ("eng", str(i.engine))

    def grant_vec(sem_id, value):
        for cum, vec in sem_hist.get(sem_id, []):
            if cum >= value:
                return vec
        return {}

    for bb in f.blocks:
        out_insts = []
        for i in bb.instructions:
            si = i.sync_info
            p = proc_of(i)
            base = dict(proc_vec.get(p, {}))
            myvec = dict(base)
            if si and si.on_wait:
                waits = list(si.on_wait)
                grants = []
                for w in waits:
                    if w.wait_mode == "sem-ge-imm" and w.wait_value is not None:
                        grants.append(grant_vec(w.id, w.wait_value))
                    else:
                        grants.append(None)  # unknown -> always keep
                cap = CAP.get(i.opcode, DEFAULT_CAP)
                if i.opcode in SKIP:
                    cap = 99
                if len(waits) > cap:
                    keep = [True] * len(waits)
                    order = sorted(range(len(waits)),
                                   key=lambda k: -(waits[k].wait_value or 0))
                    for k in order:
                        if sum(keep) <= cap:
                            break
                        if grants[k] is None:
                            continue
                        w = waits[k]
                        cov = dict(base)
                        for j2 in range(len(waits)):
                            if j2 != k and keep[j2] and grants[j2] is not None:
                                join(cov, grants[j2])
                        if cov.get(("sem", w.id), -1) >= (w.wait_value or 0):
                            keep[k] = False
                    kept = [w for k2, w in enumerate(waits) if keep[k2]]
                    if len(kept) > cap and os.environ.get("PRUNE_DEBUG"):
                        import sys
                        for wq_, gq_ in zip(waits, grants):
                            print(f"PRUNEDBG {i.name} wait {wq_.ant_name}>="
                                  f"{wq_.wait_value} grantvec="
                                  f"{ {k2: v2 for k2, v2 in (gq_ or {}).items() if isinstance(k2, tuple)} }",
                                  file=sys.stderr)
                    if len(kept) > cap:
                        # move overflow onto same-engine NoOps (<=2 each)
                        if i.opcode in ("DMACopy", "DMATranspose"):
                            import sys
                            print(f"WAITPRUNE: cannot nop-split DMA {i.name}: "
                                  f"{[(w.ant_name, w.wait_value) for w in kept]}",
                                  file=sys.stderr)
                        else:
                            overflow = kept[cap:]
                            kept = kept[:cap]
                            for c0 in range(0, len(overflow), 1):
                                chunk = overflow[c0:c0 + 1]
                                nop_n[0] += 1
                                nop = _mybir.InstNoOp(
                                    name=f"WPNOP-{nop_n[0]}",
                                    engine=i.engine,
                                    ins=[], outs=[],
                                    sync_info=_mybir.SyncInfo(
                                        on_wait=chunk, on_update=[]),
                                )
                                out_insts.append(nop)
                    si.on_wait = kept
                    waits = kept
                    grants = [grant_vec(w.id, w.wait_value)
                              if (w.wait_mode == "sem-ge-imm"
                                  and w.wait_value is not None) else None
                              for w in waits]
                for g in grants:
                    if g is not None:
                        join(myvec, g)
            # complete this instruction on proc p
            myvec[p] = myvec.get(p, 0) + 1
            if si and si.on_update:
                for u in si.on_update:
                    cum = sem_cum.get(u.id, 0) + (u.update_value or 0)
                    sem_cum[u.id] = cum
                    myvec[("sem", u.id)] = cum
                    sem_hist.setdefault(u.id, []).append((cum, dict(myvec)))
            proc_vec[p] = myvec
            out_insts.append(i)
        bb.instructions = out_insts


_BUILD_CACHE = {}


def _get_nc(seq_tiles):
    key = tuple(seq_tiles)
    if key not in _BUILD_CACHE:
        _BUILD_CACHE[key] = _build(key)
    return _BUILD_CACHE[key]


def _prepare(x, freqs_cis, seqlens, wq, wk, wv, wo):
    BF = ml_dtypes.bfloat16
    x = np.asarray(x, dtype=np.float32)
    freqs_cis = np.asarray(freqs_cis, dtype=np.float32)
    sl = np.asarray(seqlens).astype(np.int64)
    wq = np.asarray(wq, dtype=np.float32)
    wk = np.asarray(wk, dtype=np.float32)
    wv = np.asarray(wv, dtype=np.float32)
    wo = np.asarray(wo, dtype=np.float32)

    assert int(sl.sum()) == S and all(int(v) % P == 0 for v in sl)
    seq_tiles = tuple(int(v) // P for v in sl)
    nc = _get_nc(seq_tiles)

    # host-side layout prep (shared across cores)
    xT = np.ascontiguousarray(x.T.astype(BF))                # [DIM, S]
    cos = freqs_cis[:, :, 0].T                               # [64, S]
    sin = freqs_cis[:, :, 1].T
    cos2 = np.ascontiguousarray(np.concatenate([cos, cos], 0).astype(BF))
    # sign folded in, keyed by INPUT row of the shifted mul:
    # rows 0:64 = +sin (feeds upper output half), rows 64:128 = -sin
    sin2 = np.ascontiguousarray(np.concatenate([sin, -sin], 0).astype(BF))
    trimask = np.where(
        np.arange(P)[:, None] >= np.arange(P)[None, :], 0.0, NEG
    ).astype(np.float32)
    ident_np = np.eye(P, dtype=np.float32).astype(BF)
    perm = np.concatenate([np.arange(0, D, 2), np.arange(1, D, 2)])  # evens|odds

    in_maps = []
    for c in range(NCORES):
        qrows = (np.arange(HPC)[:, None] * D + c * HPC * D + perm[None, :]).ravel()
        krows = c * D + perm
        vrows = np.arange(c * D, (c + 1) * D)
        in_maps.append({
            "xT": xT,
            "wqT": np.ascontiguousarray(wq[qrows].T.astype(BF)),   # [DIM, HPC*D]
            "wkT": np.ascontiguousarray(wk[krows].T.astype(BF)),   # [DIM, D]
            "wvT": np.ascontiguousarray(wv[vrows].T.astype(BF)),   # [DIM, D]
            "woT": np.ascontiguousarray(
                wo[:, c * HPC * D:(c + 1) * HPC * D].T.astype(BF)),
            "cos2": cos2,
            "sin2": sin2,
            "trim": trimask,
            "identh": ident_np,
        })

    return nc, in_maps


def kernel(x, freqs_cis, seqlens, wq, wk, wv, wo):
    global LAST_RESULTS
    nc, in_maps = _prepare(x, freqs_cis, seqlens, wq, wk, wv, wo)
    res = run_bass_kernel_spmd(
        nc, in_maps, core_ids=list(range(NCORES)),
        trace=bool(int(os.environ.get("KERNEL_TRACE", "0"))),
    )
    LAST_RESULTS = res
    acc = res.results[0]["out"].astype(np.float32)
    for r in res.results[1:]:
        acc = acc + r["out"].astype(np.float32)
    return acc


# revision 26
# speedup vs baseline: 1.0241x; 1.0241x over previous
"""Trainium2 Bass kernel: GQA attention over packed ragged sequences.

Sharding: tensor-parallel over heads across 8 NeuronCores. Core c owns
q-heads [4c, 4c+4) and kv-head c. wq/wk/wv are sharded by head rows,
wo by columns; each core computes a full [S, DIM] partial of the output
projection and the partials are summed on the host (row-parallel
all-reduce of wo).

Layout choices:
 - All projections computed in transposed layout (feature dim on SBUF
   partitions, tokens on free dim) so the attention contractions have
   their contraction dim on partitions.
 - RoPE pairs are de-interleaved by permuting wq/wk rows on the host
   (evens then odds per head), turning RoPE into the rotate-half form:
   out = q*cos2 + roll64(q)*sin2 with the sign folded into sin2.
 - Data path is bf16 (fp32 PSUM accumulate): halves DMA + DVE cost,
   enables fast weight load, full PE rate. Output partials are stored
   bf16 and summed fp32 on host.
 - Single allocation epoch (everything fits in SBUF at bf16): no pool
   transitions. Attention for a sequence is emitted between projection
   quarters as soon as its tokens are RoPE'd; output-projection pieces
   of finished sequences are fed into attention's dependency bubbles.
"""

import os
from collections import deque
from contextlib import ExitStack
import numpy as np
import ml_dtypes

import concourse.bass as bass
import concourse.mybir as mybir
from concourse.tile import TileContext, add_dep_helper
from concourse.bass_utils import run_bass_kernel_spmd

F32 = mybir.dt.float32
BF16 = mybir.dt.bfloat16
AX = mybir.AxisListType
ALU = mybir.AluOpType
ACT = mybir.ActivationFunctionType

H, KVH, D, DIM = 32, 8, 128, 4096
NCORES = 8
HPC = H // NCORES          # q heads per core
S = 2048                   # total packed tokens
SCALE = D ** -0.5
P = 128                    # partition count / tile edge
KC = DIM // P              # contraction chunks for qkv projections
TQ, TW = 4, 512            # token quarters for projection phase
NXT = 16                   # x stream tiles (8 per alternating half)
NEG = -1.0e30

LAST_RESULTS = None        # BassKernelResults of the most recent run


def _build(seq_tiles):
    """Build the per-core Bass program. seq_tiles: tiles (of 128 tokens)
    per packed sequence, e.g. (4, 6, 2, 4).

    Sync-wait discipline (walrus caps: PE matmul LW = 1 wait, DMA = 2):
    - every DMA-produced tile is first read by a tiny PE "touch" matmul
      into a persistent [1,2] psum scratch (absorbs the DMA wait);
    - all PSUM lives in persistent tiles from one global pool;
    - _prune_waits recomputes vector clocks and drops redundant waits.
    """
    LMAX = max(seq_tiles) * P
    nseq = len(seq_tiles)
    assert LMAX <= 1024 and sum(seq_tiles) * P == S
    t0s = np.concatenate([[0], np.cumsum(seq_tiles)]).astype(int)
    # tq after whose epilogue sequence s is fully projected+RoPE'd
    ready_tq = [(int(t0s[s + 1]) * P + TW - 1) // TW - 1 for s in range(nseq)]

    nc = bass.Bass()

    xT = nc.dram_tensor("xT", [DIM, S], BF16, kind="ExternalInput")
    wqT = nc.dram_tensor("wqT", [DIM, HPC * D], BF16, kind="ExternalInput")
    wkT = nc.dram_tensor("wkT", [DIM, D], BF16, kind="ExternalInput")
    wvT = nc.dram_tensor("wvT", [DIM, D], BF16, kind="ExternalInput")
    woT = nc.dram_tensor("woT", [HPC * D, DIM], BF16, kind="ExternalInput")
    cos2 = nc.dram_tensor("cos2", [P, S], BF16, kind="ExternalInput")
    sin2 = nc.dram_tensor("sin2", [P, S], BF16, kind="ExternalInput")
    trim = nc.dram_tensor("trim", [P, P], F32, kind="ExternalInput")
    identh = nc.dram_tensor("identh", [P, P], BF16, kind="ExternalInput")
    out_d = nc.dram_tensor("out", [S, DIM], BF16, kind="ExternalOutput")

    with TileContext(nc) as tc:
        with tc.tile_pool(name="glob", bufs=1) as gp, \
             tc.tile_pool(name="globps", space="PSUM", bufs=1) as gpp, \
             tc.tile_pool(name="qkv", bufs=1, side="right") as qkvp, \
             tc.tile_pool(name="wA", bufs=1) as wp, \
             tc.tile_pool(name="xA", bufs=1) as xp, \
             tc.tile_pool(name="csA", bufs=1) as csp, \
             tc.tile_pool(name="ropeA", bufs=1) as rp, \
             tc.tile_pool(name="attP", bufs=1) as attp, \
             tc.tile_pool(name="woP", bufs=1) as wop, \
             tc.tile_pool(name="attw", bufs=1) as ap_:
            trimt = gp.tile([P, P], F32, name="trimt")
            ident = gp.tile([P, P], BF16, name="ident")
            nc.sync.dma_start(out=trimt[:, :], in_=trim[:, :])
            nc.sync.dma_start(out=ident[:, :], in_=identh[:, :])

            # q/k/v residents
            qTt = qkvp.tile([P, HPC * S], BF16, name="qTt")  # per-head [d, tok]
            kTt = qkvp.tile([P, S], BF16, name="kTt")        # [d, tok]
            vt = qkvp.tile([P, S], BF16, name="vt")          # [tok%128, blk*128+d]

            # PSUM: 3x 2-bank fp32 + two bf16 transpose banks (6, 7) with
            # an fp32 touch target aliased into bigV's tail (touches only
            # run while transpose slots hold dead data)
            big0 = gpp.tile([P, 1024], F32, name="big0")
            big1 = gpp.tile([P, 1024], F32, name="big1")
            big2 = gpp.tile([P, 1024], F32, name="big2")
            bigU = gpp.tile([P, 1024], BF16, name="bigU")
            bigV = gpp.tile([P, 1024], BF16, name="bigV")
            tps = bigV[0:1, 768:1024].bitcast(F32)

            def touch(t):
                # N=1 matmuls fail walrus's ISA check; use a [K,1]x[K,2] probe
                return nc.tensor.matmul(tps[0:1, 0:2], t[:, 0:1], t[:, 0:2],
                                        start=True, stop=True)

            touch(ident)  # absorb the identity DMA wait once

            # ---- projection-phase tiles ----
            cos2t = csp.tile([P, S], BF16, name="cos2t")
            sin2t = csp.tile([P, S], BF16, name="sin2t")
            xts = [xp.tile([P, TW], BF16, name=f"xts{i}") for i in range(NXT)]
            wq_c, wk_c, wv_c = [], [], []

            psq = [big0[:, 0:TW], big0[:, TW:2 * TW],
                   big1[:, 0:TW], big1[:, TW:2 * TW]]
            psk = big2[:, 0:TW]
            psv = big2[:, TW:2 * TW]
            pstA2 = [bigU[:, 0:P], bigV[:, 0:P]]

            traw6 = [rp.tile([P, TW], BF16, name=f"traw{i}") for i in range(5)]
            rot2 = [rp.tile([P, TW], BF16, name=f"rot{i}") for i in range(2)]
            vtmp2 = [rp.tile([P, TW], BF16, name=f"vtmp{i}") for i in range(2)]

            # ---- attention / wo tiles ----
            attT = attp.tile([P, HPC * S], BF16, name="attTt")
            Ptm2 = [ap_.tile([P, LMAX], BF16, name=f"Ptmp{i}") for i in range(4)]
            strips = [[ap_.tile([P, LMAX], BF16, name=f"strip{b}_{kb}")
                       for kb in range(max(seq_tiles))] for b in range(3)]
            se2 = [ap_.tile([P, 1], F32, name=f"se{i}") for i in range(4)]
            rr2 = [ap_.tile([P, 1], F32, name=f"rr{i}") for i in range(4)]
            idn2 = [ap_.tile([P, P], BF16, name=f"idn{i}") for i in range(4)]
            ots1 = [ap_.tile([P, 512], BF16, name=f"ots1_{i}") for i in range(4)]

            ps_s4 = [big0[:, 0:512], big0[:, 512:1024],
                     big1[:, 0:512], big1[:, 512:1024]]
            ps_sbig = [big0[:, 0:LMAX], big1[:, 0:LMAX]]
            # strip-transpose outputs are regular matmuls (fp32 psum),
            # aliased into the bf16 transpose banks
            ps_t2 = [bigU[:, 512:768].bitcast(F32),
                     bigV[:, 256:512].bitcast(F32)]

            # full wo resident (bf16 leaves room); DMA'd during tq1
            woh = [wop.tile([P, DIM], BF16, name=f"woh{f}") for f in range(HPC)]
            woh_t = [None] * HPC

            # ---- shared emission state ----
            # it: psum-slot rotation (shared by scores + wo pieces; slots
            #     are freed within one group so distance 4 is safe).
            # sc: scores-only rotation for Ptmp/Pt/se/rr, whose lifetime
            #     spans the 2-tile transpose lag — feeding wo pieces into
            #     `it` must not shorten their reuse distance.
            st_ = {"it": 0, "sc": 0, "hs": 0, "ctile": 0}
            wo_feed = deque()   # pending wo-piece callables
            feed_credit = [0.0]

            def emit_wo_piece():
                if wo_feed:
                    wo_feed.popleft()()

            def feed(rate):
                feed_credit[0] += rate
                while feed_credit[0] >= 1.0 and wo_feed:
                    feed_credit[0] -= 1.0
                    emit_wo_piece()

            def make_wo_piece(mt, nch):
                msl = slice(mt * P, (mt + 1) * P)
                nsl = slice(nch * 512, (nch + 1) * 512)

                def piece():
                    ps = ps_s4[st_["it"] % 4]
                    st_["it"] += 1
                    for f in range(HPC):
                        mi = nc.tensor.matmul(
                            ps, attT[:, f * S + mt * P: f * S + (mt + 1) * P],
                            woh[f][:, nsl], start=(f == 0), stop=(f == HPC - 1))
                        if woh_t[f] is not None:
                            add_dep_helper(mi.ins, woh_t[f].ins, sync=False,
                                           reason="woh touch first")
                            woh_t[f] = None
                    ot = ots1[st_["ctile"] % 4]
                    st_["ctile"] += 1
                    nc.scalar.copy(ot[:, :], ps)
                    nc.sync.dma_start(out=out_d[msl, nsl], in_=ot[:, :])
                return piece

            def enqueue_wo(s):
                for mt in range(int(t0s[s]), int(t0s[s + 1])):
                    for nch in range(DIM // 512):
                        wo_feed.append(make_wo_piece(mt, nch))

            def attn_seq(s, rate):
                """Emit attention for sequence s. Yields between PE groups;
                interleaves wo-feed pieces at the given rate per group.
                Software-pipelined: transposes for a tile are emitted two
                score-tiles later so PE has independent work while the
                exp/normalize chain completes."""
                T = int(seq_tiles[s])
                kb0 = int(t0s[s]) * P
                LAGN = 2

                def scores_tile(h, qt):
                    L = (qt + 1) * P
                    q0 = kb0 + qt * P
                    ps_s = (ps_s4[st_["it"] % 4] if L <= 512
                            else ps_sbig[st_["it"] % 2])
                    st_["it"] += 1
                    b = st_["sc"] % 4
                    st_["sc"] += 1
                    Ptmp = Ptm2[b]
                    qsl = qTt[:, h * S + q0: h * S + q0 + P]
                    off = 0
                    while off < L:
                        w = min(512, L - off)
                        nc.tensor.matmul(ps_s[:, off:off + w], qsl,
                                         kTt[:, kb0 + off: kb0 + off + w],
                                         start=True, stop=True)
                        off += w
                    nc.vector.tensor_add(ps_s[:, L - P:L], ps_s[:, L - P:L],
                                         trimt[:, :])
                    # no max-subtraction: |scale*s| < ~25 keeps exp finite.
                    # Normalization rides the strip transposes: build
                    # diag(1/se) and use it as the "identity" of a regular
                    # transposing matmul — Ptmp^T @ diag(rr) both reorients
                    # and normalizes, removing the [P, L] normalize op from
                    # the per-tile chain.
                    nc.scalar.activation(Ptmp[:, :L], ps_s[:, :L], ACT.Exp,
                                         scale=SCALE, accum_out=se2[b][:, 0:1])
                    nc.vector.reciprocal(rr2[b][:, :], se2[b][:, :])
                    nc.vector.tensor_scalar_mul(idn2[b][:, :], ident[:, :],
                                                rr2[b][:, 0:1])
                    return b

                def transposes_tile(sb, qt, b):
                    Ptmp = Ptm2[b]
                    for kb in range(qt + 1):
                        ps_t = ps_t2[kb % 2]
                        nc.tensor.matmul(ps_t, Ptmp[:, kb * P:(kb + 1) * P],
                                         idn2[b][:, :], start=True, stop=True)
                        nc.vector.tensor_copy(
                            sb[kb][:, qt * P:(qt + 1) * P], ps_t)

                for h in range(HPC):
                    sb = strips[st_["hs"] % 3]
                    st_["hs"] += 1
                    pend = deque()
                    for qt in range(T):
                        b = scores_tile(h, qt)
                        pend.append((qt, b))
                        feed(rate)
                        yield
                        if len(pend) > LAGN:
                            qt_, b_ = pend.popleft()
                            transposes_tile(sb, qt_, b_)
                            feed(rate)
                            yield
                    while pend:
                        qt_, b_ = pend.popleft()
                        transposes_tile(sb, qt_, b_)
                        feed(rate)
                        yield
                    W = T * P
                    # PV over the transposed strips: out[d, q-range]
                    for c0 in range(0, W, 512):
                        c1 = min(c0 + 512, W)
                        nkb = c1 // P
                        for kb in range(nkb):
                            r0 = max(c0, kb * P)
                            kt0 = kb0 + kb * P
                            nc.tensor.matmul(big2[:, r0:c1],
                                             vt[:, kt0:kt0 + P],
                                             sb[kb][:, r0:c1],
                                             start=(kb == 0),
                                             stop=(kb == nkb - 1))
                        feed(rate)
                        yield
                    nc.vector.tensor_copy(
                        attT[:, h * S + kb0: h * S + kb0 + W], big2[:, 0:W])
                    feed(rate)
                    yield

            def run_gen(g):
                for _ in g:
                    pass

            def rope_finish(traw, rot, dest, tsl):
                """dest = traw*cos2 + roll64(traw)*sin2 (sign in sin2)."""
                nc.vector.tensor_mul(rot[0:64, :], traw[64:128, :],
                                     sin2t[64:128, tsl])
                nc.vector.tensor_mul(rot[64:128, :], traw[0:64, :],
                                     sin2t[0:64, tsl])
                nc.vector.tensor_mul(traw[:, :], traw[:, :], cos2t[:, tsl])
                nc.vector.tensor_add(dest, traw[:, :], rot[:, :])

            # ================= main emission =================
            for tq in range(TQ):
                tsl = slice(tq * TW, (tq + 1) * TW)
                for k in range(KC):
                    xt = xts[k % 8 + 8 * (tq % 2)]
                    nc.sync.dma_start(out=xt[:, :],
                                      in_=xT[k * P:(k + 1) * P, tsl])
                    if tq == 0:
                        wq_k = wp.tile([P, HPC * D], BF16, name=f"wq{k}")
                        wk_k = wp.tile([P, D], BF16, name=f"wk{k}")
                        wv_k = wp.tile([P, D], BF16, name=f"wv{k}")
                        ksl = slice(k * P, (k + 1) * P)
                        nc.sync.dma_start(out=wq_k[:, :], in_=wqT[ksl, :])
                        nc.sync.dma_start(out=wk_k[:, :], in_=wkT[ksl, :])
                        nc.sync.dma_start(out=wv_k[:, :], in_=wvT[ksl, :])
                        wq_c.append(wq_k)
                        wk_c.append(wk_k)
                        wv_c.append(wv_k)
                        if k == 16:
                            nc.sync.dma_start(out=cos2t[:, :], in_=cos2[:, :])
                            nc.sync.dma_start(out=sin2t[:, :], in_=sin2[:, :])
                    if tq == 1 and k % 8 == 0:
                        # stream the wo slab in during tq1 (x traffic halves
                        # after tq0's weights are done)
                        f = k // 8
                        nc.sync.dma_start(out=woh[f][:, :],
                                          in_=woT[f * P:(f + 1) * P, :])
                    need_touch = (tq == 0) or (k == 0)
                    ti = touch(xt) if need_touch else None
                    st, sp = (k == 0), (k == KC - 1)
                    mms = []
                    for h in range(HPC):
                        mms.append(nc.tensor.matmul(
                            psq[h], wq_c[k][:, h * D:(h + 1) * D],
                            xt[:, :], start=st, stop=sp))
                    mms.append(nc.tensor.matmul(
                        psk, wk_c[k][:, :], xt[:, :], start=st, stop=sp))
                    mms.append(nc.tensor.matmul(
                        psv, wv_c[k][:, :], xt[:, :], start=st, stop=sp))
                    if ti is not None:
                        for mi in mms:
                            add_dep_helper(mi.ins, ti.ins, sync=False,
                                           reason="xt touch first")
                # epilogue: drain psum banks with copies split across ACT
                # and DVE (banks 0-3 are needed by the attention segment
                # right after); RoPE itself is deferred past the segment —
                # its results aren't needed until a later segment.
                vtmp = vtmp2[tq % 2]
                nc.scalar.copy(vtmp[:, :], psv)
                nc.scalar.copy(traw6[0][:, :], psq[0])
                nc.vector.tensor_copy(traw6[1][:, :], psq[1])
                nc.scalar.copy(traw6[2][:, :], psq[2])
                nc.vector.tensor_copy(traw6[3][:, :], psq[3])
                for j in range(TW // P):
                    pstA = pstA2[j % 2]
                    nc.tensor.transpose(pstA, vtmp[:, j * P:(j + 1) * P],
                                        ident[:, :])
                    tok = tq * TW + j * P
                    nc.scalar.copy(vt[:, tok:tok + P], pstA)
                nc.vector.tensor_copy(traw6[4][:, :], psk)

                def emit_rope(tq_):
                    tsl_ = slice(tq_ * TW, (tq_ + 1) * TW)
                    for h in range(HPC):
                        rope_finish(
                            traw6[h], rot2[h % 2],
                            qTt[:, h * S + tq_ * TW: h * S + (tq_ + 1) * TW],
                            tsl_)
                    rope_finish(traw6[4], rot2[0], kTt[:, tsl_], tsl_)

                # ---- post-tq attention / wo segments ----
                if tq == 0:
                    emit_rope(0)
                if tq == 1:
                    for s in range(nseq):
                        if ready_tq[s] <= 0:
                            run_gen(attn_seq(s, 0.0))
                            enqueue_wo(s)
                    emit_rope(1)
                if tq == 2:
                    for f in range(HPC):
                        woh_t[f] = touch(woh[f])
                    # drain most of seq0's wo here; keep some as feed for
                    # the first post-tq3 attention segment
                    while len(wo_feed) > 24:
                        emit_wo_piece()
                    emit_rope(2)
            # after the last tq: remaining sequences, feeding wo pieces of
            # finished sequences into attention bubbles, then drain.
            # tq3's RoPE is emitted after the first segment (it is only
            # needed by the last sequence's attention).
            done = [s for s in range(nseq) if ready_tq[s] <= 0]
            rope3 = [False]
            for s in range(nseq):
                if s in done:
                    continue
                npieces = max(1, len(wo_feed))
                ngroups = (2 * int(seq_tiles[s]) + 3) * HPC
                rate = min(3.0, npieces / ngroups)
                run_gen(attn_seq(s, rate))
                enqueue_wo(s)
                if not rope3[0]:
                    emit_rope(3)
                    rope3[0] = True
            if not rope3[0]:
                emit_rope(3)
            while wo_feed:
                emit_wo_piece()

    _prune_waits(nc)
    return nc


def _prune_waits(nc):
    """Fit instructions into walrus's per-instruction sync-command budget
    (~2 commands: waits + updates; matmul LW and DMA take 1 wait).

    Tile's stage-1B emits each instruction's full required vector clock as
    waits when the executing proc hasn't observed the ticks, without
    transitive minimization. We recompute exact vector clocks over the
    emitted sem graph (procs execute in program order; engines and DMA
    lanes complete in FIFO order), drop waits implied by the proc's
    predecessor or by other kept waits' grants, and move any genuine
    overflow onto injected same-engine NoOps just before the instruction.
    """
    import concourse.mybir as _mybir

    f = nc.m.functions[0]

    CAP = {}
    SKIP = {"NoOp", "EventSemaphore", "AllEngineBarrier", "Halt"}
    DEFAULT_CAP = 1

    def join(a, b):
        for k, v in b.items():
            if a.get(k, -1) < v:
                a[k] = v
        return a

    sem_hist = {}    # sem id -> list of (cum_value_after, vec_of_updater)
    sem_cum = {}     # sem id -> cumulative value
    proc_vec = {}    # proc key -> vector of last completed inst on proc
    nop_n = [0]

    def proc_of(i):
        si = i.sync_info
        if i.opcode in ("DMACopy", "DMATranspose") and si and si.on_update:
            return ("sem", si.on_update[0].id)
        return ("eng", str(i.engine))

    def grant_vec(sem_id, value):
        for cum, vec in sem_hist.get(sem_id, []):
            if cum >= value:
                return vec
        return {}

    for bb in f.blocks:
        out_insts = []
        for i in bb.instructions:
            si = i.sync_info
            p = proc_of(i)
            base = dict(proc_vec.get(p, {}))
            myvec = dict(base)
            if si and si.on_wait:
                waits = list(si.on_wait)
                grants = []
                for w in waits:
                    if w.wait_mode == "sem-ge-imm" and w.wait_value is not None:
                        grants.append(grant_vec(w.id, w.wait_value))
                    else:
                        grants.append(None)  # unknown -> always keep
                cap = CAP.get(i.opcode, DEFAULT_CAP)
                if i.opcode in SKIP:
                    cap = 99
                if len(waits) > cap:
                    keep = [True] * len(waits)
                    order = sorted(range(len(waits)),
                                   key=lambda k: -(waits[k].wait_value or 0))
                    for k in order:
                        if sum(keep) <= cap:
                            break
                        if grants[k] is None:
                            continue
                        w = waits[k]
                        cov = dict(base)
                        for j2 in range(len(waits)):
                            if j2 != k and keep[j2] and grants[j2] is not None:
                                join(cov, grants[j2])
                        if cov.get(("sem", w.id), -1) >= (w.wait_value or 0):
                            keep[k] = False
                    kept = [w for k2, w in enumerate(waits) if keep[k2]]
                    if len(kept) > cap and os.environ.get("PRUNE_DEBUG"):
                        import sys
                        for wq_, gq_ in zip(waits, grants):
                            print(f"PRUNEDBG {i.name} wait {wq_.ant_name}>="
                                  f"{wq_.wait_value} grantvec="
                                  f"{ {k2: v2 for k2, v2 in (gq_ or {}).items() if isinstance(k2, tuple)} }",
                                  file=sys.stderr)
                    if len(kept) > cap:
                        # move overflow onto same-engine NoOps (<=2 each)
                        if i.opcode in ("DMACopy", "DMATranspose"):
                            import sys
                            print(f"WAITPRUNE: cannot nop-split DMA {i.name}: "
                                  f"{[(w.ant_name, w.wait_value) for w in kept]}",
                                  file=sys.stderr)
                        else:
                            overflow = kept[cap:]
                            kept = kept[:cap]
                            for c0 in range(0, len(overflow), 1):
                                chunk = overflow[c0:c0 + 1]
                                nop_n[0] += 1
                                nop = _mybir.InstNoOp(
                                    name=f"WPNOP-{nop_n[0]}",
                                    engine=i.engine,
                                    ins=[], outs=[],
                                    sync_info=_mybir.SyncInfo(
                                        on_wait=chunk, on_update=[]),
                                )
                                out_insts.append(nop)
                    si.on_wait = kept
                    waits = kept
                    grants = [grant_vec(w.id, w.wait_value)
                              if (w.wait_mode == "sem-ge-imm"
                                  and w.wait_value is not None) else None
                              for w in waits]
                for g in grants:
                    if g is not None:
                        join(myvec, g)
            # complete this instruction on proc p
            myvec[p] = myvec.get(p, 0) + 1
            if si and si.on_update:
                for u in si.on_update:
                    cum = sem_cum.get(u.id, 0) + (u.update_value or 0)
                    sem_cum[u.id] = cum
                    myvec[("sem", u.id)] = cum
                    sem_hist.setdefault(u.id, []).append((cum, dict(myvec)))
            proc_vec[p] = myvec
            out_insts.append(i)
        bb.instructions = out_insts


_BUILD_CACHE = {}


def _get_nc(seq_tiles):
    key = tuple(seq_tiles)
    if key not in _BUILD_CACHE:
        _BUILD_CACHE[key] = _build(key)
    return _BUILD_CACHE[key]


def _prepare(x, freqs_cis, seqlens, wq, wk, wv, wo):
    BF = ml_dtypes.bfloat16
    x = np.asarray(x, dtype=np.float32)
    freqs_cis = np.asarray(freqs_cis, dtype=np.float32)
    sl = np.asarray(seqlens).astype(np.int64)
    wq = np.asarray(wq, dtype=np.float32)
    wk = np.asarray(wk, dtype=np.float32)
    wv = np.asarray(wv, dtype=np.float32)
    wo = np.asarray(wo, dtype=np.float32)

    assert int(sl.sum()) == S and all(int(v) % P == 0 for v in sl)
    seq_tiles = tuple(int(v) // P for v in sl)
    nc = _get_nc(seq_tiles)

    # host-side layout prep (shared across cores)
    xT = np.ascontiguousarray(x.T.astype(BF))                # [DIM, S]
    cos = freqs_cis[:, :, 0].T                               # [64, S]
    sin = freqs_cis[:, :, 1].T
    cos2 = np.ascontiguousarray(np.concatenate([cos, cos], 0).astype(BF))
    # sign folded in, keyed by INPUT row of the shifted mul:
    # rows 0:64 = +sin (feeds upper output half), rows 64:128 = -sin
    sin2 = np.ascontiguousarray(np.concatenate([sin, -sin], 0).astype(BF))
    trimask = np.where(
        np.arange(P)[:, None] >= np.arange(P)[None, :], 0.0, NEG
    ).astype(np.float32)
    ident_np = np.eye(P, dtype=np.float32).astype(BF)
    perm = np.concatenate([np.arange(0, D, 2), np.arange(1, D, 2)])  # evens|odds

    in_maps = []
    for c in range(NCORES):
        qrows = (np.arange(HPC)[:, None] * D + c * HPC * D + perm[None, :]).ravel()
        krows = c * D + perm
        vrows = np.arange(c * D, (c + 1) * D)
        in_maps.append({
            "xT": xT,
            "wqT": np.ascontiguousarray(wq[qrows].T.astype(BF)),   # [DIM, HPC*D]
            "wkT": np.ascontiguousarray(wk[krows].T.astype(BF)),   # [DIM, D]
            "wvT": np.ascontiguousarray(wv[vrows].T.astype(BF)),   # [DIM, D]
            "woT": np.ascontiguousarray(
                wo[:, c * HPC * D:(c + 1) * HPC * D].T.astype(BF)),
            "cos2": cos2,
            "sin2": sin2,
            "trim": trimask,
            "identh": ident_np,
        })

    return nc, in_maps


def kernel(x, freqs_cis, seqlens, wq, wk, wv, wo):
    global LAST_RESULTS
    nc, in_maps = _prepare(x, freqs_cis, seqlens, wq, wk, wv, wo)
    res = run_bass_kernel_spmd(
        nc, in_maps, core_ids=list(range(NCORES)),
        trace=bool(int(os.environ.get("KERNEL_TRACE", "0"))),
    )
    LAST_RESULTS = res
    acc = res.results[0]["out"].astype(np.float32)
    for r in res.results[1:]:
        acc = acc + r["out"].astype(np.float32)
    return acc


# revision 33
# speedup vs baseline: 1.1772x; 1.1495x over previous
"""Trainium2 Bass kernel: GQA attention over packed ragged sequences.

Sharding: tensor-parallel over heads across 8 NeuronCores. Core c owns
q-heads [4c, 4c+4) and kv-head c. wq/wk/wv are sharded by head rows,
wo by columns; each core computes a full [S, DIM] partial of the output
projection and the partials are summed on the host (row-parallel
all-reduce of wo).

Layout choices:
 - All projections computed in transposed layout (feature dim on SBUF
   partitions, tokens on free dim) so the attention contractions have
   their contraction dim on partitions.
 - RoPE pairs are de-interleaved by permuting wq/wk rows on the host
   (evens then odds per head), turning RoPE into the rotate-half form:
   out = q*cos2 + roll64(q)*sin2 with the sign folded into sin2.
 - Data path is bf16 (fp32 PSUM accumulate): halves DMA + DVE cost,
   enables fast weight load, full PE rate. Output partials are stored
   bf16 and summed fp32 on host.
 - Single allocation epoch (everything fits in SBUF at bf16): no pool
   transitions. Attention for a sequence is emitted between projection
   quarters as soon as its tokens are RoPE'd; output-projection pieces
   of finished sequences are fed into attention's dependency bubbles.
"""

import os
from collections import deque
from contextlib import ExitStack
import numpy as np
import ml_dtypes

import concourse.bass as bass
import concourse.mybir as mybir
from concourse.tile import TileContext, add_dep_helper
from concourse.bass_utils import run_bass_kernel_spmd

F32 = mybir.dt.float32
BF16 = mybir.dt.bfloat16
AX = mybir.AxisListType
ALU = mybir.AluOpType
ACT = mybir.ActivationFunctionType

H, KVH, D, DIM = 32, 8, 128, 4096
NCORES = 8
HPC = H // NCORES          # q heads per core
S = 2048                   # total packed tokens
SCALE = D ** -0.5
P = 128                    # partition count / tile edge
KC = DIM // P              # contraction chunks for qkv projections
TQ, TW = 4, 512            # token quarters for projection phase
NXT = 16                   # x stream tiles (8 per alternating half)
NEG = -1.0e30

LAST_RESULTS = None        # BassKernelResults of the most recent run


def _build(seq_tiles):
    """Build the per-core Bass program. seq_tiles: tiles (of 128 tokens)
    per packed sequence, e.g. (4, 6, 2, 4).

    Sync-wait discipline (walrus caps: PE matmul LW = 1 wait, DMA = 2):
    - every DMA-produced tile is first read by a tiny PE "touch" matmul
      into a persistent [1,2] psum scratch (absorbs the DMA wait);
    - all PSUM lives in persistent tiles from one global pool;
    - _prune_waits recomputes vector clocks and drops redundant waits.
    """
    LMAX = max(seq_tiles) * P
    nseq = len(seq_tiles)
    assert LMAX <= 1024 and sum(seq_tiles) * P == S
    t0s = np.concatenate([[0], np.cumsum(seq_tiles)]).astype(int)
    # tq after whose epilogue sequence s is fully projected+RoPE'd
    ready_tq = [(int(t0s[s + 1]) * P + TW - 1) // TW - 1 for s in range(nseq)]

    nc = bass.Bass()

    xT = nc.dram_tensor("xT", [DIM, S], BF16, kind="ExternalInput")
    wqT = nc.dram_tensor("wqT", [DIM, HPC * D], BF16, kind="ExternalInput")
    wkT = nc.dram_tensor("wkT", [DIM, D], BF16, kind="ExternalInput")
    wvT = nc.dram_tensor("wvT", [DIM, D], BF16, kind="ExternalInput")
    woT = nc.dram_tensor("woT", [HPC * D, DIM], BF16, kind="ExternalInput")
    cos2 = nc.dram_tensor("cos2", [P, S], BF16, kind="ExternalInput")
    sin2 = nc.dram_tensor("sin2", [P, S], BF16, kind="ExternalInput")
    trim = nc.dram_tensor("trim", [P, P], F32, kind="ExternalInput")
    identh = nc.dram_tensor("identh", [P, P], BF16, kind="ExternalInput")
    out_d = nc.dram_tensor("out", [S, DIM], BF16, kind="ExternalOutput")

    with TileContext(nc) as tc:
        with tc.tile_pool(name="glob", bufs=1) as gp, \
             tc.tile_pool(name="globps", space="PSUM", bufs=1) as gpp, \
             tc.tile_pool(name="qkv", bufs=1, side="right") as qkvp, \
             tc.tile_pool(name="wA", bufs=1) as wp, \
             tc.tile_pool(name="xA", bufs=1) as xp, \
             tc.tile_pool(name="csA", bufs=1) as csp, \
             tc.tile_pool(name="ropeA", bufs=1) as rp, \
             tc.tile_pool(name="attP", bufs=1) as attp, \
             tc.tile_pool(name="woP", bufs=1) as wop, \
             tc.tile_pool(name="attw", bufs=1) as ap_:
            trimt = gp.tile([P, P], F32, name="trimt")
            ident = gp.tile([P, P], BF16, name="ident")
            nc.sync.dma_start(out=trimt[:, :], in_=trim[:, :])
            nc.sync.dma_start(out=ident[:, :], in_=identh[:, :])

            # q/k/v residents
            qTt = qkvp.tile([P, HPC * S], BF16, name="qTt")  # per-head [d, tok]
            kTt = qkvp.tile([P, S], BF16, name="kTt")        # [d, tok]
            vt = qkvp.tile([P, S], BF16, name="vt")          # [tok%128, blk*128+d]

            # PSUM: 3x 2-bank fp32 + two bf16 transpose banks (6, 7) with
            # an fp32 touch target aliased into bigV's tail (touches only
            # run while transpose slots hold dead data)
            big0 = gpp.tile([P, 1024], F32, name="big0")
            big1 = gpp.tile([P, 1024], F32, name="big1")
            big2 = gpp.tile([P, 1024], F32, name="big2")
            bigU = gpp.tile([P, 1024], BF16, name="bigU")
            bigV = gpp.tile([P, 1024], BF16, name="bigV")
            tps = bigV[0:1, 768:1024].bitcast(F32)

            def touch(t):
                # N=1 matmuls fail walrus's ISA check; use a [K,1]x[K,2] probe
                return nc.tensor.matmul(tps[0:1, 0:2], t[:, 0:1], t[:, 0:2],
                                        start=True, stop=True)

            touch(ident)  # absorb the identity DMA wait once

            # ---- projection-phase tiles ----
            cos2t = csp.tile([P, S], BF16, name="cos2t")
            sin2t = csp.tile([P, S], BF16, name="sin2t")
            xts = [xp.tile([P, TW], BF16, name=f"xts{i}") for i in range(NXT)]
            wq_c, wk_c, wv_c = [], [], []

            psq = [big0[:, 0:TW], big0[:, TW:2 * TW],
                   big1[:, 0:TW], big1[:, TW:2 * TW]]
            psk = big2[:, 0:TW]
            psv = big2[:, TW:2 * TW]
            pstA2 = [bigU[:, 0:P], bigV[:, 0:P]]

            traw6 = [rp.tile([P, TW], BF16, name=f"traw{i}") for i in range(5)]
            rot2 = [rp.tile([P, TW], BF16, name=f"rot{i}") for i in range(2)]
            vtmp2 = [rp.tile([P, TW], BF16, name=f"vtmp{i}") for i in range(2)]

            # ---- attention / wo tiles ----
            attT = attp.tile([P, HPC * S], BF16, name="attTt")
            Ptm2 = [ap_.tile([P, LMAX], BF16, name=f"Ptmp{i}") for i in range(4)]
            strips = [[ap_.tile([P, LMAX], BF16, name=f"strip{b}_{kb}")
                       for kb in range(max(seq_tiles))] for b in range(4)]
            se2 = [ap_.tile([P, 1], F32, name=f"se{i}") for i in range(4)]
            rr2 = [ap_.tile([P, 1], F32, name=f"rr{i}") for i in range(4)]
            idn2 = [ap_.tile([P, P], BF16, name=f"idn{i}") for i in range(4)]
            ots1 = [ap_.tile([P, 512], BF16, name=f"ots1_{i}") for i in range(4)]

            ps_s4 = [big0[:, 0:512], big0[:, 512:1024],
                     big1[:, 0:512], big1[:, 512:1024]]
            ps_sbig = [big0[:, 0:LMAX], big1[:, 0:LMAX]]
            # strip-transpose outputs are regular matmuls (fp32 psum),
            # aliased into the bf16 transpose banks
            ps_t2 = [bigU[:, 512:768].bitcast(F32),
                     bigV[:, 256:512].bitcast(F32)]

            # full wo resident (bf16 leaves room); DMA'd during tq1
            woh = [wop.tile([P, DIM], BF16, name=f"woh{f}") for f in range(HPC)]
            woh_t = [None] * HPC

            # ---- shared emission state ----
            # it: psum-slot rotation (shared by scores + wo pieces; slots
            #     are freed within one group so distance 4 is safe).
            # sc: scores-only rotation for Ptmp/Pt/se/rr, whose lifetime
            #     spans the 2-tile transpose lag — feeding wo pieces into
            #     `it` must not shorten their reuse distance.
            st_ = {"it": 0, "sc": 0, "hs": 0, "ctile": 0}
            wo_feed = deque()   # pending wo-piece callables
            feed_credit = [0.0]

            def emit_wo_piece():
                if wo_feed:
                    wo_feed.popleft()()

            def feed(rate):
                feed_credit[0] += rate
                while feed_credit[0] >= 1.0 and wo_feed:
                    feed_credit[0] -= 1.0
                    emit_wo_piece()

            def make_wo_piece(mt, nch):
                msl = slice(mt * P, (mt + 1) * P)
                nsl = slice(nch * 512, (nch + 1) * 512)

                def piece():
                    ps = ps_s4[st_["it"] % 4]
                    st_["it"] += 1
                    for f in range(HPC):
                        mi = nc.tensor.matmul(
                            ps, attT[:, f * S + mt * P: f * S + (mt + 1) * P],
                            woh[f][:, nsl], start=(f == 0), stop=(f == HPC - 1))
                        if woh_t[f] is not None:
                            add_dep_helper(mi.ins, woh_t[f].ins, sync=False,
                                           reason="woh touch first")
                            woh_t[f] = None
                    ot = ots1[st_["ctile"] % 4]
                    st_["ctile"] += 1
                    if st_["ctile"] % 2 == 0:
                        nc.scalar.copy(ot[:, :], ps)
                    else:
                        nc.vector.tensor_copy(ot[:, :], ps)
                    nc.sync.dma_start(out=out_d[msl, nsl], in_=ot[:, :])
                return piece

            def enqueue_wo(s):
                for mt in range(int(t0s[s]), int(t0s[s + 1])):
                    for nch in range(DIM // 512):
                        wo_feed.append(make_wo_piece(mt, nch))

            def attn_seq(s, rate, g):
                """Emit attention for sequence s on generator slot g (0/1).
                Yields between PE groups; two generators are interleaved so
                each provides the other's independent PE work while its own
                exp chain completes, plus wo-feed pieces at `rate`/group.
                Per-slot resources: sc slots {2g, 2g+1}, strip buffer, and
                PV accumulator bank (big2 half) so interleaved PV chains
                never share a psum bank's has_written state."""
                T = int(seq_tiles[s])
                kb0 = int(t0s[s]) * P
                LAGN = 1
                scc = [0]
                pv = big2[:, g * 512:(g + 1) * 512]

                def scores_tile(h, qt):
                    L = (qt + 1) * P
                    q0 = kb0 + qt * P
                    ps_s = (ps_s4[st_["it"] % 4] if L <= 512
                            else ps_sbig[st_["it"] % 2])
                    st_["it"] += 1
                    b = 2 * g + scc[0] % 2
                    scc[0] += 1
                    Ptmp = Ptm2[b]
                    qsl = qTt[:, h * S + q0: h * S + q0 + P]
                    off = 0
                    while off < L:
                        w = min(512, L - off)
                        nc.tensor.matmul(ps_s[:, off:off + w], qsl,
                                         kTt[:, kb0 + off: kb0 + off + w],
                                         start=True, stop=True)
                        off += w
                    nc.vector.tensor_add(ps_s[:, L - P:L], ps_s[:, L - P:L],
                                         trimt[:, :])
                    # no max-subtraction: |scale*s| < ~25 keeps exp finite.
                    # Normalization rides the strip transposes: build
                    # diag(1/se) and use it as the "identity" of a regular
                    # transposing matmul — Ptmp^T @ diag(rr) both reorients
                    # and normalizes, removing the [P, L] normalize op from
                    # the per-tile chain.
                    nc.scalar.activation(Ptmp[:, :L], ps_s[:, :L], ACT.Exp,
                                         scale=SCALE, accum_out=se2[b][:, 0:1])
                    nc.vector.reciprocal(rr2[b][:, :], se2[b][:, :])
                    nc.vector.tensor_scalar_mul(idn2[b][:, :], ident[:, :],
                                                rr2[b][:, 0:1])
                    return b

                sb = strips[g]

                def transposes_tile(qt, b):
                    Ptmp = Ptm2[b]
                    for kb in range(qt + 1):
                        ps_t = ps_t2[kb % 2]
                        nc.tensor.matmul(ps_t, Ptmp[:, kb * P:(kb + 1) * P],
                                         idn2[b][:, :], start=True, stop=True)
                        nc.vector.tensor_copy(
                            sb[kb][:, qt * P:(qt + 1) * P], ps_t)

                for h in range(HPC):
                    sb = strips[g + 2 * (h % 2)]
                    pend = deque()
                    for qt in range(T):
                        b = scores_tile(h, qt)
                        pend.append((qt, b))
                        feed(rate)
                        yield
                        if len(pend) > LAGN:
                            qt_, b_ = pend.popleft()
                            transposes_tile(qt_, b_)
                            feed(rate)
                            yield
                    while pend:
                        qt_, b_ = pend.popleft()
                        transposes_tile(qt_, b_)
                        feed(rate)
                        yield
                    W = T * P
                    # PV over the transposed strips, chunked through this
                    # generator's psum bank; attT receives each chunk
                    for c0 in range(0, W, 512):
                        c1 = min(c0 + 512, W)
                        nkb = c1 // P
                        for kb in range(nkb):
                            r0 = max(c0, kb * P)
                            kt0 = kb0 + kb * P
                            nc.tensor.matmul(pv[:, r0 - c0:c1 - c0],
                                             vt[:, kt0:kt0 + P],
                                             sb[kb][:, r0:c1],
                                             start=(kb == 0),
                                             stop=(kb == nkb - 1))
                        feed(rate)
                        yield
                        nc.vector.tensor_copy(
                            attT[:, h * S + kb0 + c0: h * S + kb0 + c1],
                            pv[:, 0:c1 - c0])
                        feed(rate)
                        yield

            def run_pair(ga, gb):
                """Round-robin two generators until both are exhausted."""
                gens = deque([x for x in (ga, gb) if x is not None])
                while gens:
                    gx = gens.popleft()
                    try:
                        next(gx)
                        gens.append(gx)
                    except StopIteration:
                        pass

            def rope_finish(traw, rot, dest, tsl):
                """dest = traw*cos2 + roll64(traw)*sin2 (sign in sin2)."""
                nc.vector.tensor_mul(rot[0:64, :], traw[64:128, :],
                                     sin2t[64:128, tsl])
                nc.vector.tensor_mul(rot[64:128, :], traw[0:64, :],
                                     sin2t[0:64, tsl])
                nc.vector.tensor_mul(traw[:, :], traw[:, :], cos2t[:, tsl])
                nc.vector.tensor_add(dest, traw[:, :], rot[:, :])

            # ================= main emission =================
            for tq in range(TQ):
                tsl = slice(tq * TW, (tq + 1) * TW)
                for k in range(KC):
                    xt = xts[k % 8 + 8 * (tq % 2)]
                    nc.sync.dma_start(out=xt[:, :],
                                      in_=xT[k * P:(k + 1) * P, tsl])
                    if tq == 0:
                        wq_k = wp.tile([P, HPC * D], BF16, name=f"wq{k}")
                        wk_k = wp.tile([P, D], BF16, name=f"wk{k}")
                        wv_k = wp.tile([P, D], BF16, name=f"wv{k}")
                        ksl = slice(k * P, (k + 1) * P)
                        nc.sync.dma_start(out=wq_k[:, :], in_=wqT[ksl, :])
                        nc.sync.dma_start(out=wk_k[:, :], in_=wkT[ksl, :])
                        nc.sync.dma_start(out=wv_k[:, :], in_=wvT[ksl, :])
                        wq_c.append(wq_k)
                        wk_c.append(wk_k)
                        wv_c.append(wv_k)
                        if k == 16:
                            nc.sync.dma_start(out=cos2t[:, :], in_=cos2[:, :])
                            nc.sync.dma_start(out=sin2t[:, :], in_=sin2[:, :])
                    if tq == 1 and k % 8 == 0:
                        # stream the wo slab in during tq1 (x traffic halves
                        # after tq0's weights are done)
                        f = k // 8
                        nc.sync.dma_start(out=woh[f][:, :],
                                          in_=woT[f * P:(f + 1) * P, :])
                    need_touch = (tq == 0) or (k == 0)
                    ti = touch(xt) if need_touch else None
                    st, sp = (k == 0), (k == KC - 1)
                    mms = []
                    for h in range(HPC):
                        mms.append(nc.tensor.matmul(
                            psq[h], wq_c[k][:, h * D:(h + 1) * D],
                            xt[:, :], start=st, stop=sp))
                    mms.append(nc.tensor.matmul(
                        psk, wk_c[k][:, :], xt[:, :], start=st, stop=sp))
                    mms.append(nc.tensor.matmul(
                        psv, wv_c[k][:, :], xt[:, :], start=st, stop=sp))
                    if ti is not None:
                        for mi in mms:
                            add_dep_helper(mi.ins, ti.ins, sync=False,
                                           reason="xt touch first")
                # epilogue: drain psum banks with copies split across ACT
                # and DVE (banks 0-3 are needed by the attention segment
                # right after); RoPE itself is deferred past the segment —
                # its results aren't needed until a later segment.
                vtmp = vtmp2[tq % 2]
                nc.scalar.copy(vtmp[:, :], psv)
                nc.scalar.copy(traw6[0][:, :], psq[0])
                nc.vector.tensor_copy(traw6[1][:, :], psq[1])
                nc.scalar.copy(traw6[2][:, :], psq[2])
                nc.vector.tensor_copy(traw6[3][:, :], psq[3])
                for j in range(TW // P):
                    pstA = pstA2[j % 2]
                    nc.tensor.transpose(pstA, vtmp[:, j * P:(j + 1) * P],
                                        ident[:, :])
                    tok = tq * TW + j * P
                    nc.scalar.copy(vt[:, tok:tok + P], pstA)
                nc.vector.tensor_copy(traw6[4][:, :], psk)

                def emit_rope(tq_):
                    tsl_ = slice(tq_ * TW, (tq_ + 1) * TW)
                    for h in range(HPC):
                        rope_finish(
                            traw6[h], rot2[h % 2],
                            qTt[:, h * S + tq_ * TW: h * S + (tq_ + 1) * TW],
                            tsl_)
                    rope_finish(traw6[4], rot2[0], kTt[:, tsl_], tsl_)

                if tq < TQ - 1:
                    emit_rope(tq)
            # ---- tail: all attention + output projection ----
            # The projection quarters stay dense back-to-back (a sparse
            # attention segment between them trips the HAM throttle for
            # the following dense phase). Attention runs as interleaved
            # sequence-pairs: each generator's groups are the other's
            # independent PE work; wo pieces of finished sequences feed
            # the second pair; the wo stream drains at the end.
            order = sorted(range(nseq), key=lambda s: ready_tq[s])
            for f in range(HPC):
                woh_t[f] = touch(woh[f])
            run_pair(attn_seq(order[0], 0.0, 0),
                     attn_seq(order[1], 0.0, 1) if nseq > 1 else None)
            emit_rope(TQ - 1)
            enqueue_wo(order[0])
            if nseq > 1:
                enqueue_wo(order[1])
            if nseq > 2:
                n_grp = sum((2 * int(seq_tiles[s]) + 4) * HPC
                            for s in order[2:4])
                rate = min(3.0, len(wo_feed) / max(1, 2 * n_grp))
                run_pair(attn_seq(order[2], rate, 0),
                         attn_seq(order[3], rate, 1) if nseq > 3 else None)
                for s in order[2:]:
                    enqueue_wo(s)
            while wo_feed:
                emit_wo_piece()

    _prune_waits(nc)
    return nc


def _prune_waits(nc):
    """Fit instructions into walrus's per-instruction sync-command budget
    (~2 commands: waits + updates; matmul LW and DMA take 1 wait).

    Tile's stage-1B emits each instruction's full required vector clock as
    waits when the executing proc hasn't observed the ticks, without
    transitive minimization. We recompute exact vector clocks over the
    emitted sem graph (procs execute in program order; engines and DMA
    lanes complete in FIFO order), drop waits implied by the proc's
    predecessor or by other kept waits' grants, and move any genuine
    overflow onto injected same-engine NoOps just before the instruction.
    """
    import concourse.mybir as _mybir

    f = nc.m.functions[0]

    CAP = {}
    SKIP = {"NoOp", "EventSemaphore", "AllEngineBarrier", "Halt"}
    DEFAULT_CAP = 1

    def join(a, b):
        for k, v in b.items():
            if a.get(k, -1) < v:
                a[k] = v
        return a

    sem_hist = {}    # sem id -> list of (cum_value_after, vec_of_updater)
    sem_cum = {}     # sem id -> cumulative value
    proc_vec = {}    # proc key -> vector of last completed inst on proc
    nop_n = [0]

    def proc_of(i):
        si = i.sync_info
        if i.opcode in ("DMACopy", "DMATranspose") and si and si.on_update:
            return ("sem", si.on_update[0].id)
        return ("eng", str(i.engine))

    def grant_vec(sem_id, value):
        for cum, vec in sem_hist.get(sem_id, []):
            if cum >= value:
                return vec
        return {}

    for bb in f.blocks:
        out_insts = []
        for i in bb.instructions:
            si = i.sync_info
            p = proc_of(i)
            base = dict(proc_vec.get(p, {}))
            myvec = dict(base)
            if si and si.on_wait:
                waits = list(si.on_wait)
                grants = []
                for w in waits:
                    if w.wait_mode == "sem-ge-imm" and w.wait_value is not None:
                        grants.append(grant_vec(w.id, w.wait_value))
                    else:
                        grants.append(None)  # unknown -> always keep
                cap = CAP.get(i.opcode, DEFAULT_CAP)
                if i.opcode in SKIP:
                    cap = 99
                if len(waits) > cap:
                    keep = [True] * len(waits)
                    order = sorted(range(len(waits)),
                                   key=lambda k: -(waits[k].wait_value or 0))
                    for k in order:
                        if sum(keep) <= cap:
                            break
                        if grants[k] is None:
                            continue
                        w = waits[k]
                        cov = dict(base)
                        for j2 in range(len(waits)):
                            if j2 != k and keep[j2] and grants[j2] is not None:
                                join(cov, grants[j2])
                        if cov.get(("sem", w.id), -1) >= (w.wait_value or 0):
                            keep[k] = False
                    kept = [w for k2, w in enumerate(waits) if keep[k2]]
                    if len(kept) > cap and os.environ.get("PRUNE_DEBUG"):
                        import sys
                        for wq_, gq_ in zip(waits, grants):
                            print(f"PRUNEDBG {i.name} wait {wq_.ant_name}>="
                                  f"{wq_.wait_value} grantvec="
                                  f"{ {k2: v2 for k2, v2 in (gq_ or {}).items() if isinstance(k2, tuple)} }",
                                  file=sys.stderr)
                    if len(kept) > cap:
                        # move overflow onto same-engine NoOps (<=2 each)
                        if i.opcode in ("DMACopy", "DMATranspose"):
                            import sys
                            print(f"WAITPRUNE: cannot nop-split DMA {i.name}: "
                                  f"{[(w.ant_name, w.wait_value) for w in kept]}",
                                  file=sys.stderr)
                        else:
                            overflow = kept[cap:]
                            kept = kept[:cap]
                            for c0 in range(0, len(overflow), 1):
                                chunk = overflow[c0:c0 + 1]
                                nop_n[0] += 1
                                nop = _mybir.InstNoOp(
                                    name=f"WPNOP-{nop_n[0]}",
                                    engine=i.engine,
                                    ins=[], outs=[],
                                    sync_info=_mybir.SyncInfo(
                                        on_wait=chunk, on_update=[]),
                                )
                                out_insts.append(nop)
                    si.on_wait = kept
                    waits = kept
                    grants = [grant_vec(w.id, w.wait_value)
                              if (w.wait_mode == "sem-ge-imm"
                                  and w.wait_value is not None) else None
                              for w in waits]
                for g in grants:
                    if g is not None:
                        join(myvec, g)
            # complete this instruction on proc p
            myvec[p] = myvec.get(p, 0) + 1
            if si and si.on_update:
                for u in si.on_update:
                    cum = sem_cum.get(u.id, 0) + (u.update_value or 0)
                    sem_cum[u.id] = cum
                    myvec[("sem", u.id)] = cum
                    sem_hist.setdefault(u.id, []).append((cum, dict(myvec)))
            proc_vec[p] = myvec
            out_insts.append(i)
        bb.instructions = out_insts


_BUILD_CACHE = {}


def _get_nc(seq_tiles):
    key = tuple(seq_tiles)
    if key not in _BUILD_CACHE:
        _BUILD_CACHE[key] = _build(key)
    return _BUILD_CACHE[key]


def _prepare(x, freqs_cis, seqlens, wq, wk, wv, wo):
    BF = ml_dtypes.bfloat16
    x = np.asarray(x, dtype=np.float32)
    freqs_cis = np.asarray(freqs_cis, dtype=np.float32)
    sl = np.asarray(seqlens).astype(np.int64)
    wq = np.asarray(wq, dtype=np.float32)
    wk = np.asarray(wk, dtype=np.float32)
    wv = np.asarray(wv, dtype=np.float32)
    wo = np.asarray(wo, dtype=np.float32)

    assert int(sl.sum()) == S and all(int(v) % P == 0 for v in sl)
    seq_tiles = tuple(int(v) // P for v in sl)
    nc = _get_nc(seq_tiles)

    # host-side layout prep (shared across cores)
    xT = np.ascontiguousarray(x.T.astype(BF))                # [DIM, S]
    cos = freqs_cis[:, :, 0].T                               # [64, S]
    sin = freqs_cis[:, :, 1].T
    cos2 = np.ascontiguousarray(np.concatenate([cos, cos], 0).astype(BF))
    # sign folded in, keyed by INPUT row of the shifted mul:
    # rows 0:64 = +sin (feeds upper output half), rows 64:128 = -sin
    sin2 = np.ascontiguousarray(np.concatenate([sin, -sin], 0).astype(BF))
    trimask = np.where(
        np.arange(P)[:, None] >= np.arange(P)[None, :], 0.0, NEG
    ).astype(np.float32)
    ident_np = np.eye(P, dtype=np.float32).astype(BF)
    perm = np.concatenate([np.arange(0, D, 2), np.arange(1, D, 2)])  # evens|odds

    in_maps = []
    for c in range(NCORES):
        qrows = (np.arange(HPC)[:, None] * D + c * HPC * D + perm[None, :]).ravel()
        krows = c * D + perm
        vrows = np.arange(c * D, (c + 1) * D)
        in_maps.append({
            "xT": xT,
            "wqT": np.ascontiguousarray(wq[qrows].T.astype(BF)),   # [DIM, HPC*D]
            "wkT": np.ascontiguousarray(wk[krows].T.astype(BF)),   # [DIM, D]
            "wvT": np.ascontiguousarray(wv[vrows].T.astype(BF)),   # [DIM, D]
            "woT": np.ascontiguousarray(
                wo[:, c * HPC * D:(c + 1) * HPC * D].T.astype(BF)),
            "cos2": cos2,
            "sin2": sin2,
            "trim": trimask,
            "identh": ident_np,
        })

    return nc, in_maps


def kernel(x, freqs_cis, seqlens, wq, wk, wv, wo):
    global LAST_RESULTS
    nc, in_maps = _prepare(x, freqs_cis, seqlens, wq, wk, wv, wo)
    res = run_bass_kernel_spmd(
        nc, in_maps, core_ids=list(range(NCORES)),
        trace=bool(int(os.environ.get("KERNEL_TRACE", "0"))),
    )
    LAST_RESULTS = res
    acc = res.results[0]["out"].astype(np.float32)
    for r in res.results[1:]:
        acc = acc + r["out"].astype(np.float32)
    return acc
